# revision 35
# baseline (speedup 1.0000x reference)
"""Multi-head attention (B=2, S=2048, D=1024, H=16) on 8 TRN2 NeuronCores.

Sharding: tensor-parallel over heads x data-parallel over batch.
Core c handles batch b = c//4, head group g = c%4 (4 heads, 256 cols).
W_q/W_k/W_v are split column-wise per group, W_o row-wise; each core
produces a partial [S, D] output, reduced on the host (the W_o
contraction is a pure sum over head groups; b_v/b_o folded in on host).

Device kernel (per core), all matmuls bf16 with fp32 PSUM accumulation:
  - K^T, Q^T projections in transposed layout [dk*2, S] (lhsT = W cols,
    rhs = x^T), V in natural layout [S, dk*4+ones] (lhsT = x^T chunks).
  - scores computed transposed: ST[k,q] = (K^T chunk)^T-matmul vs Q^T.
    The two heads of a pair (rows 0-63 / 64-127 of KT/QT) are issued
    back-to-back as PE row-tiles (tile_position (0,0) / (64,0)) so the
    hardware streams them concurrently -- logits cost ~1 matmul slot
    per head pair instead of 2.
  - softmax without max-subtraction (logits are O(5), exp is safe):
    exp on ACT straight out of PSUM with scale=1/sqrt(dk), one
    [128, 1024] activation per (pair, chunk) covering both heads.
    lgx/lgy psum tiles ping-pong so ACT back-pressure never blocks
    the PE fill of the next chunk.
  - ctx^T[dk+1, q] accumulated over k-chunks with an ones-augmented V
    (row dk = softmax denominators), normalized via DVE with a gpsimd
    partition-broadcast of the reciprocals.
  - out partial = ctx^T-chunks @ W_o rows, accumulated over the 2
    128-row chunks of the group's 256 W_o rows.
"""

import numpy as np
import ml_dtypes
from contextlib import ExitStack

import concourse.bass as bass
import concourse.tile as tile
from concourse import bacc, mybir
from concourse.bass_utils import run_bass_kernel_spmd

BF16 = mybir.dt.bfloat16
F32 = mybir.dt.float32

D = 1024            # model dim
H = 16              # heads
DK = 64             # head dim
NCORES = 8
GPB = 4             # head groups per batch (= cores per batch)
HPG = H // GPB      # 4 heads per core
HD = HPG * DK       # 256 cols per group
HAUG = DK + 1       # 65: head block width in augmented-V layout
SP = 512            # q-span / free-dim tile
SCALE = 1.0 / np.sqrt(DK)


def build(S, debug_dump=False):
    NQS = S // SP       # q spans
    NSC = S // 128      # sequence chunks (k side)
    NDC = D // 128      # model-dim chunks
    SI = SP // 128      # s-chunks per q-span

    nc = bacc.Bacc("TRN2", target_bir_lowering=False, debug=False)
    HA = HPG * HAUG     # 260: augmented V width
    xT_e = nc.dram_tensor("xT", [S // SP, 128, D // 128, SP], BF16, kind="ExternalInput")
    wq_e = nc.dram_tensor("wq", [128, 2, D // 128, 128], BF16, kind="ExternalInput")
    wk_e = nc.dram_tensor("wk", [128, 2, D // 128, 128], BF16, kind="ExternalInput")
    wv_e = nc.dram_tensor("wv", [128, D // 128, HD], BF16, kind="ExternalInput")
    wo_e = nc.dram_tensor("wo", [128, 2, D], BF16, kind="ExternalInput")
    bq_e = nc.dram_tensor("bq", [128, 2], F32, kind="ExternalInput")
    bk_e = nc.dram_tensor("bk", [128, 2], F32, kind="ExternalInput")
    ones_e = nc.dram_tensor("ones", [128, HPG], BF16, kind="ExternalInput")
    out_e = nc.dram_tensor("out", [S, D], F32, kind="ExternalOutput")

    ADD = mybir.AluOpType.add
    MULT = mybir.AluOpType.mult
    EXP = mybir.ActivationFunctionType.Exp

    with tile.TileContext(nc) as tc, ExitStack() as ctx:
        const = ctx.enter_context(tc.tile_pool(name="const", bufs=1))
        qpool = ctx.enter_context(tc.tile_pool(name="qpool", bufs=2))
        cpool = ctx.enter_context(tc.tile_pool(name="cpool", bufs=2))
        ptp = ctx.enter_context(tc.tile_pool(name="ptp", bufs=6))
        obp = ctx.enter_context(tc.tile_pool(name="obp", bufs=4))
        smp = ctx.enter_context(tc.tile_pool(name="smp", bufs=3))
        accp = ctx.enter_context(tc.tile_pool(name="accp", bufs=4))
        # lgx/lgy: 2 banks each, single-buffered -> X/Y ping-pong
        plg = ctx.enter_context(tc.tile_pool(name="plg", bufs=1, space="PSUM"))
        # ctx accumulators + wo/proj scratch, 1 bank each, double-buffered
        pacc = ctx.enter_context(tc.tile_pool(name="pacc", bufs=2, space="PSUM"))

        wq_sb = const.tile([128, 2, NDC, 128], BF16, name="wq_sb")
        wk_sb = const.tile([128, 2, NDC, 128], BF16, name="wk_sb")
        wv_sb = const.tile([128, NDC, HD], BF16, name="wv_sb")
        onesP = const.tile([128, HPG], BF16, name="onesP")
        wo_sb = const.tile([128, 2, D], BF16, name="wo_sb")
        bq_sb = const.tile([128, 2], F32, name="bq_sb")
        bk_sb = const.tile([128, 2], F32, name="bk_sb")
        xT_sb = [const.tile([128, NDC, SP], BF16, name=f"xT{q}") for q in range(NQS)]
        KT_sb = const.tile([128, 2, S], BF16, name="KT_sb")
        V_sb = const.tile([128, NSC, HD], BF16, name="V_sb")

        # input DMAs: host pre-tiles everything to the exact SBUF layout,
        # so each tensor is one flat contiguous transfer. All inputs ride
        # one dynamic queue in emission order, so the order below is the
        # arrival order the prologue compute is scheduled against. The
        # m=0 halves of W_k/W_q come first: job 0 only needs those.
        nc.sync.dma_start(bk_sb[:], bk_e.ap())
        nc.sync.dma_start(bq_sb[:], bq_e.ap())
        nc.sync.dma_start(onesP[:], ones_e.ap())
        nc.sync.dma_start(wk_sb[:, 0, 0:2, :], wk_e.ap()[:, 0, 0:2, :])
        nc.sync.dma_start(xT_sb[0][:, 0:2, :], xT_e.ap()[0, :, 0:2, :])
        nc.sync.dma_start(wk_sb[:, 0, 2:, :], wk_e.ap()[:, 0, 2:, :])
        nc.sync.dma_start(xT_sb[0][:, 2:, :], xT_e.ap()[0, :, 2:, :])
        nc.sync.dma_start(wq_sb[:, 0], wq_e.ap()[:, 0])
        nc.sync.dma_start(wv_sb[:], wv_e.ap())
        nc.sync.dma_start(wq_sb[:, 1], wq_e.ap()[:, 1])
        nc.sync.dma_start(wk_sb[:, 1], wk_e.ap()[:, 1])
        for q in range(1, NQS):
            nc.sync.dma_start(xT_sb[q][:], xT_e.ap()[q])
        nc.sync.dma_start(wo_sb[:], wo_e.ap())

        # K^T projection group: KT[128 (2 heads), m, s]
        def emit_kproj_group(m, q):
            ps = pacc.tile([128, SP], F32, tag="wo", name="kps")
            for c in range(NDC):
                nc.tensor.matmul(
                    ps[:], wk_sb[:, m, c, :],
                    xT_sb[q][:, c, :],
                    start=(c == 0), stop=(c == NDC - 1))
            nc.vector.tensor_scalar(
                KT_sb[:, m, q * SP:(q + 1) * SP], ps[:],
                bk_sb[:, m:m + 1], None, ADD)

        # V projection, natural layout [s-chunk, 4*64]
        def emit_vproj_group(sc):
            q, si = divmod(sc, SI)
            ps = pacc.tile([128, HD], F32, tag="wo", name="vps")
            for c in range(NDC):
                nc.tensor.matmul(
                    ps[:], xT_sb[q][:, c, si * 128:(si + 1) * 128],
                    wv_sb[:, c, :],
                    start=(c == 0), stop=(c == NDC - 1))
            nc.vector.tensor_copy(V_sb[:, sc, :], ps[:])

        # last-span W_o is two-pass: m0 partials land in these persistent
        # SBUF tiles mid-span, only m1 + add + DMA remain for the epilogue
        woa = [const.tile([128, SP], F32, name=f"woa{i}")
               for i in range(SI * (D // SP))]

        def make_qproj_parts(QTn, qsrc, m, nparts=2):
            cell = []
            step = NDC // nparts
            def part(p):
                def run():
                    if p == 0:
                        cell.append(pacc.tile([128, SP], F32, tag="wo",
                                              name="qps"))
                    ps = cell[0]
                    for c in range(p * step, (p + 1) * step):
                        nc.tensor.matmul(
                            ps[:], wq_sb[:, m, c, :],
                            xT_sb[qsrc][:, c, :],
                            start=(c == 0), stop=(c == NDC - 1))
                    if p == nparts - 1:
                        nc.vector.tensor_scalar(
                            QTn[:, m, :], ps[:], bq_sb[:, m:m + 1], None, ADD)
                return run
            return [part(p) for p in range(nparts)]

        def emit_qproj_group(QTn, qsrc, m):
            for run in make_qproj_parts(QTn, qsrc, m, nparts=1):
                run()

        # Logits for one (pair, chunk): the two heads' K=64 matmuls go to
        # PE row groups 0/64 back-to-back (concurrent in HW), then one
        # [128, 1024] exp covers both heads.
        def emit_lg_pair(QT, p, scp, j, tag):
            lg = plg.tile([128, 2 * SP], F32, tag=tag, name="lg")
            sc = 2 * scp + j
            for hh in range(2):
                r = hh * 64
                nc.tensor.matmul(
                    lg[:, hh * SP:(hh + 1) * SP],
                    KT_sb[r:r + 64, p, sc * 128:(sc + 1) * 128],
                    QT[r:r + 64, p, :],
                    start=True, stop=True)
            pt = ptp.tile([128, 2 * SP], BF16, name="pt")
            nc.scalar.activation(pt[:], lg[:], EXP, scale=float(SCALE))
            return pt

        # ctx for one pair job: the two heads are PE col-tiles writing
        # partitions 0-63 / 64-127 of one shared psum bank (concurrent in
        # HW), and the softmax denominators accumulate on the DVE in bf16.
        def emit_ctx_pair(CT, cps_by_p, acc_by_p, p, scp, ptX, ptY):
            if scp == 0:
                cps_by_p[p] = pacc.tile([128, SP], F32, tag="ctx",
                                        name="cps")
                acc_by_p[p] = accp.tile([128, 2 * SP], BF16, name="acc")
            cps = cps_by_p[p]
            acc = acc_by_p[p]
            for j, pt in ((0, ptX), (1, ptY)):
                sc = 2 * scp + j
                for hh in range(2):
                    h = 2 * p + hh
                    nc.tensor.matmul(
                        cps[hh * 64:(hh + 1) * 64, :],
                        V_sb[:, sc, h * DK:(h + 1) * DK],
                        pt[:, hh * SP:(hh + 1) * SP],
                        start=(sc == 0), stop=(sc == NSC - 1))
            # denominator partials: full-width [128, 1024] adds (both
            # heads at once) halve the DVE op count vs per-head slices
            if scp == 0:
                nc.vector.tensor_tensor(acc[:], ptX[:], ptY[:], ADD)
            else:
                nc.vector.tensor_tensor(acc[:], acc[:], ptX[:], ADD)
                nc.vector.tensor_tensor(acc[:], acc[:], ptY[:], ADD)
            if scp == NSC // 2 - 1:
                # both reduce+recip+broadcast parts run before either
                # multiply; all at +1 so every CT write is emitted before
                # any consumer (wo from jidx>=2, passa from jidx>=10)
                na = emit_norm(CT, 2 * p, cps, acc[:, 0:SP])
                nb = emit_norm(CT, 2 * p + 1, cps, acc[:, SP:2 * SP])
                return [(1, na[0]), (1, nb[0]), (1, na[1]), (1, nb[1])]
            return None

        def emit_norm(CT, h, cps, acc):
            # denominators: PE partition-reduce of the bf16 accumulator
            # (ones lhsT, M=4 to stay off degenerate-shape paths), then
            # DVE reciprocal + gpsimd partition-broadcast a job ahead of
            # the multiply, keeping ~1us latency off the W_o path.
            cell = []
            def run_recip():
                nt = pacc.tile([68, SP], F32, tag="wo", name="nt")
                nc.tensor.matmul(nt[64:68, :], onesP[:, 0:4], acc[:],
                                 start=True, stop=True)
                sm = smp.tile([1, SP], F32, name="sm")
                nc.vector.tensor_copy(sm[:], nt[64:65, :])
                rc = smp.tile([1, SP], F32, name="rc")
                nc.vector.reciprocal_approx_fast(rc[:], sm[:])
                bc = smp.tile([64, SP], F32, name="bc")
                nc.gpsimd.partition_broadcast(bc[:], rc[:])
                cell.append(bc)
            def run_rest():
                m, hh = divmod(h, 2)
                r = hh * 64
                nc.vector.tensor_tensor(
                    CT[r:r + 64, m, :], cps[r:r + 64, :], cell[0][:], MULT)
            return run_recip, run_rest

        def make_wo_ops(q, CT):
            # each (si, dh) group split into two single-matmul halves so the
            # interleave never adds more than one extra matmul per period
            ops = []
            for si in range(SI):
                sc = SI * q + si
                for dh in range(D // SP):
                    cell = []
                    def half_a(si=si, dh=dh, CT=CT, cell=cell):
                        cell.append(pacc.tile([128, SP], F32, tag="wo",
                                              name="po"))
                        nc.tensor.matmul(
                            cell[0][:], CT[:, 0, si * 128:(si + 1) * 128],
                            wo_sb[:, 0, dh * SP:(dh + 1) * SP],
                            start=True, stop=False)
                    def half_b(si=si, sc=sc, dh=dh, CT=CT, cell=cell):
                        po = cell[0]
                        nc.tensor.matmul(
                            po[:], CT[:, 1, si * 128:(si + 1) * 128],
                            wo_sb[:, 1, dh * SP:(dh + 1) * SP],
                            start=False, stop=True)
                        ob = obp.tile([128, SP], F32, name="ob")
                        nc.vector.tensor_copy(ob[:], po[:])
                        nc.sync.dma_start(
                            out_e.ap()[sc * 128:(sc + 1) * 128,
                                       dh * SP:(dh + 1) * SP], ob[:])
                    ops.append(half_a)
                    ops.append(half_b)
            return ops

        # prologue: only what span-0 job 0 needs -- K^T (m=0, span 0) and
        # Q^T m=0. Everything else (remaining K^T, all of V, Q^T m=1)
        # interleaves into span 0's jobs just-in-time against DMA arrival.
        emit_kproj_group(0, 0)

        JPS = 2 * (NSC // 2)            # jobs per span: (pair, scp)
        QT_t = {0: qpool.tile([128, 2, SP], BF16, name="QT")}
        emit_qproj_group(QT_t[0], 0, 0)
        CT_t = {}
        cps_t = {}
        acc_t = {}
        LAG = 2
        pend_q = []                     # [(q, p, scp, ptX, ptY)]
        deferred = []                   # (due_gidx, closure)
        wo_queue = []
        # span-0 fill work, per-job lists: each producer is emitted at
        # latest one job before (or, for KT, in the same job but ahead
        # of) its first consumer, roughly in DMA-arrival order.
        vp = lambda sc: (lambda: emit_vproj_group(sc))
        kp = lambda q: (lambda: emit_kproj_group(1, q))
        k0 = lambda q: (lambda: emit_kproj_group(0, q))
        fill_sched = [
            [vp(0), vp(1), lambda: emit_qproj_group(QT_t[0], 0, 1)],
            [k0(1), vp(2)],
            [vp(3), vp(4)],
            [k0(2), vp(5)],
            [vp(6), vp(7)],
            [k0(3), vp(8), vp(9)],
            [vp(10), vp(11)],
            [kp(0), vp(12), vp(13)],
            [vp(14), vp(15)],
            [kp(1)],
            [kp(2)],
            [kp(3)],
        ]

        for q in range(NQS):
            CT_t[q] = cpool.tile([128, 2, SP], BF16, name="CT")
            cps_t[q] = {}
            acc_t[q] = {}
            qpart_queue = []
            if q + 1 < NQS:
                QT_t[q + 1] = qpool.tile([128, 2, SP], BF16, name="QT")
                qpart_queue = (make_qproj_parts(QT_t[q + 1], q + 1, 0,
                                                nparts=NDC)
                               + make_qproj_parts(QT_t[q + 1], q + 1, 1,
                                                  nparts=NDC))

            # in the last span, pair 1 runs first so its heads' norms
            # (CT[:, 1, :]) complete mid-span -- pass-a covers the m=1
            # W_o rows then, leaving only m=0 + add + DMA for the tail
            p_of = (lambda jj: 1 - jj // (NSC // 2)) if q == NQS - 1 \
                else (lambda jj: jj // (NSC // 2))
            pa_m = 1
            passa = []
            if q == NQS - 1:
                def mk_passa(i, si, dh, CT=CT_t[q]):
                    def run():
                        po = pacc.tile([128, SP], F32, tag="wo", name="po")
                        nc.tensor.matmul(
                            po[:], CT[:, pa_m, si * 128:(si + 1) * 128],
                            wo_sb[:, pa_m, dh * SP:(dh + 1) * SP],
                            start=True, stop=True)
                        nc.vector.tensor_copy(woa[i][:], po[:])
                    return run
                passa = [mk_passa(si * (D // SP) + dh, si, dh)
                         for si in range(SI) for dh in range(D // SP)]

            for jidx in range(JPS):
                gidx = q * JPS + jidx
                p, scp = p_of(jidx), jidx % (NSC // 2)
                # norms first: they free the ctx psum slots this job's
                # ctx matmuls may be waiting to reuse
                for ent in list(deferred):
                    if ent[0] <= gidx:
                        ent[1]()
                        deferred.remove(ent)
                ptX = emit_lg_pair(QT_t[q], p, scp, 0, "lgx")
                ptY = emit_lg_pair(QT_t[q], p, scp, 1, "lgy")
                pend_q.append((q, p, scp, ptX, ptY))
                # span-0 producers: after this job's logits (which only
                # read earlier products) but before the lagged ctx
                if q == 0 and jidx < len(fill_sched):
                    for op in fill_sched[jidx]:
                        op()
                if len(pend_q) > LAG:
                    ent0 = pend_q.pop(0)
                    nrm = emit_ctx_pair(CT_t[ent0[0]], cps_t[ent0[0]],
                                        acc_t[ent0[0]], *ent0[1:])
                    if nrm is not None:
                        for delta, closure in nrm:
                            deferred.append((gidx + delta, closure))
                if wo_queue and jidx >= 2:
                    wo_queue.pop(0)()
                    if wo_queue and jidx % 2 == 0:
                        wo_queue.pop(0)()
                if qpart_queue and jidx >= 4:
                    qpart_queue.pop(0)()
                    if qpart_queue:
                        qpart_queue.pop(0)()
                if passa and jidx >= 10:
                    passa.pop(0)()
                    if passa:
                        passa.pop(0)()
            if q < NQS - 1:
                wo_queue.extend(make_wo_ops(q, CT_t[q]))

        # epilogue: drain the pipeline
        norms = []
        for ent0 in pend_q:
            nrm = emit_ctx_pair(CT_t[ent0[0]], cps_t[ent0[0]],
                                acc_t[ent0[0]], *ent0[1:])
            if nrm is not None:
                norms.extend(nrm)
        for ent in sorted(deferred, key=lambda e: e[0]):
            ent[1]()
        for _, closure in sorted(norms, key=lambda e: e[0]):
            closure()
        for op in wo_queue:
            op()
        for op in passa:
            op()
        CTl = CT_t[NQS - 1]
        ep_m = 1 - pa_m
        for si in range(SI):
            sc = SI * (NQS - 1) + si
            for dh in range(D // SP):
                i = si * (D // SP) + dh
                po = pacc.tile([128, SP], F32, tag="wo", name="po")
                nc.tensor.matmul(
                    po[:], CTl[:, ep_m, si * 128:(si + 1) * 128],
                    wo_sb[:, ep_m, dh * SP:(dh + 1) * SP],
                    start=True, stop=True)
                ob = obp.tile([128, SP], F32, name="ob")
                nc.vector.tensor_tensor(ob[:], woa[i][:], po[:], ADD)
                nc.sync.dma_start(
                    out_e.ap()[sc * 128:(sc + 1) * 128,
                               dh * SP:(dh + 1) * SP], ob[:])

    nc.compile()
    return nc


_NC_CACHE = {}


def get_nc(S):
    if S not in _NC_CACHE:
        _NC_CACHE[S] = build(S)
    return _NC_CACHE[S]


def make_in_maps(x, W_q, b_q, W_k, b_k, W_v, b_v, W_o, b_o):
    B, S, _ = x.shape
    bf = ml_dtypes.bfloat16
    in_maps = []
    for core in range(NCORES):
        b, g = divmod(core, GPB)
        sl = slice(HD * g, HD * (g + 1))
        def wtile(w):
            # [D, N] -> [128, D//128, N] partition-major chunk layout
            return np.ascontiguousarray(
                np.asarray(w).reshape(D // 128, 128, -1).transpose(1, 0, 2))
        def wtile_m(w):
            # [D, 256] -> [128, 2(m), D//128, 128] m-major chunk layout
            return np.ascontiguousarray(
                np.asarray(w).reshape(D // 128, 128, 2, 128)
                .transpose(1, 2, 0, 3))
        in_maps.append({
            "xT": np.ascontiguousarray(
                np.asarray(x[b]).T.reshape(D // 128, 128, S // SP, SP)
                .transpose(2, 1, 0, 3)).astype(bf),
            "wq": wtile_m(W_q[:, sl]).astype(bf),
            "wk": wtile_m(W_k[:, sl]).astype(bf),
            "wv": wtile(W_v[:, sl]).astype(bf),
            "wo": np.ascontiguousarray(
                np.asarray(W_o[sl, :]).reshape(2, 128, D)
                .transpose(1, 0, 2)).astype(bf),
            "bq": np.ascontiguousarray(
                np.asarray(b_q[sl]).reshape(2, 128).T).astype(np.float32),
            "bk": np.ascontiguousarray(
                np.asarray(b_k[sl]).reshape(2, 128).T).astype(np.float32),
            "ones": np.ones((128, HPG), dtype=bf),
        })
    return in_maps


def unshard(results, x, W_o, b_v, b_o):
    B, S, _ = x.shape
    out = np.zeros((B, S, D), np.float32)
    for core in range(NCORES):
        b = core // GPB
        out[b] += results[core]["out"]
    const = np.asarray(b_v).astype(np.float64) @ np.asarray(W_o).astype(np.float64)
    const += np.asarray(b_o).astype(np.float64)
    out += const.astype(np.float32)[None, None, :]
    return out


def run(inputs, trace=False):
    x = np.asarray(inputs["x"])
    nc = get_nc(x.shape[1])
    in_maps = make_in_maps(
        x, inputs["W_q"], inputs["b_q"], inputs["W_k"], inputs["b_k"],
        inputs["W_v"], inputs["b_v"], inputs["W_o"], inputs["b_o"])
    def attempt():
        res = run_bass_kernel_spmd(
            nc, in_maps, core_ids=list(range(NCORES)), trace=trace)
        # force materialization here: PJRT surfaces device errors lazily
        res.results = [{k: np.asarray(v) for k, v in r.items()}
                       for r in res.results]
        return res
    try:
        res = attempt()
    except Exception:
        # transient device errors (e.g. NRT_EXEC_UNIT_UNRECOVERABLE) clear
        # on re-execution of the same NEFF
        res = attempt()
    out = unshard(res.results, x, inputs["W_o"], inputs["b_v"], inputs["b_o"])
    return out, res


def kernel(**inputs):
    out, _ = run(inputs, trace=False)
    return out


# revision 36
# speedup vs baseline: 1.0257x; 1.0257x over previous
"""Multi-head attention (B=2, S=2048, D=1024, H=16) on 8 TRN2 NeuronCores.

Sharding: tensor-parallel over heads x data-parallel over batch.
Core c handles batch b = c//4, head group g = c%4 (4 heads, 256 cols).
W_q/W_k/W_v are split column-wise per group, W_o row-wise; each core
produces a partial [S, D] output, reduced on the host (the W_o
contraction is a pure sum over head groups; b_v/b_o folded in on host).

Device kernel (per core), all matmuls bf16 with fp32 PSUM accumulation:
  - K^T, Q^T projections in transposed layout [dk*2, S] (lhsT = W cols,
    rhs = x^T), V in natural layout [S, dk*4+ones] (lhsT = x^T chunks).
  - scores computed transposed: ST[k,q] = (K^T chunk)^T-matmul vs Q^T.
    The two heads of a pair (rows 0-63 / 64-127 of KT/QT) are issued
    back-to-back as PE row-tiles (tile_position (0,0) / (64,0)) so the
    hardware streams them concurrently -- logits cost ~1 matmul slot
    per head pair instead of 2.
  - softmax without max-subtraction (logits are O(5), exp is safe):
    exp on ACT straight out of PSUM with scale=1/sqrt(dk), one
    [128, 1024] activation per (pair, chunk) covering both heads.
    lgx/lgy psum tiles ping-pong so ACT back-pressure never blocks
    the PE fill of the next chunk.
  - ctx^T[dk+1, q] accumulated over k-chunks with an ones-augmented V
    (row dk = softmax denominators), normalized via DVE with a gpsimd
    partition-broadcast of the reciprocals.
  - out partial = ctx^T-chunks @ W_o rows, accumulated over the 2
    128-row chunks of the group's 256 W_o rows.
"""

import numpy as np
import ml_dtypes
from contextlib import ExitStack

import concourse.bass as bass
import concourse.tile as tile
from concourse import bacc, mybir
from concourse.bass_utils import run_bass_kernel_spmd

BF16 = mybir.dt.bfloat16
F32 = mybir.dt.float32

D = 1024            # model dim
H = 16              # heads
DK = 64             # head dim
NCORES = 8
GPB = 4             # head groups per batch (= cores per batch)
HPG = H // GPB      # 4 heads per core
HD = HPG * DK       # 256 cols per group
HAUG = DK + 1       # 65: head block width in augmented-V layout
SP = 512            # q-span / free-dim tile
SCALE = 1.0 / np.sqrt(DK)


def build(S, debug_dump=False):
    NQS = S // SP       # q spans
    NSC = S // 128      # sequence chunks (k side)
    NDC = D // 128      # model-dim chunks
    SI = SP // 128      # s-chunks per q-span

    nc = bacc.Bacc("TRN2", target_bir_lowering=False, debug=False)
    HA = HPG * HAUG     # 260: augmented V width
    xT_e = nc.dram_tensor("xT", [S // SP, 128, D // 128, SP], BF16, kind="ExternalInput")
    wq_e = nc.dram_tensor("wq", [128, 2, D // 128, 128], BF16, kind="ExternalInput")
    wk_e = nc.dram_tensor("wk", [128, 2, D // 128, 128], BF16, kind="ExternalInput")
    wv_e = nc.dram_tensor("wv", [128, D // 128, HD], BF16, kind="ExternalInput")
    wo_e = nc.dram_tensor("wo", [128, 2, D], BF16, kind="ExternalInput")
    bq_e = nc.dram_tensor("bq", [128, 2], F32, kind="ExternalInput")
    bk_e = nc.dram_tensor("bk", [128, 2], F32, kind="ExternalInput")
    ones_e = nc.dram_tensor("ones", [128, HPG], BF16, kind="ExternalInput")
    out_e = nc.dram_tensor("out", [S, D], F32, kind="ExternalOutput")

    ADD = mybir.AluOpType.add
    MULT = mybir.AluOpType.mult
    EXP = mybir.ActivationFunctionType.Exp

    with tile.TileContext(nc) as tc, ExitStack() as ctx:
        const = ctx.enter_context(tc.tile_pool(name="const", bufs=1))
        qpool = ctx.enter_context(tc.tile_pool(name="qpool", bufs=2))
        cpool = ctx.enter_context(tc.tile_pool(name="cpool", bufs=2))
        ptp = ctx.enter_context(tc.tile_pool(name="ptp", bufs=6))
        obp = ctx.enter_context(tc.tile_pool(name="obp", bufs=4))
        smp = ctx.enter_context(tc.tile_pool(name="smp", bufs=3))
        accp = ctx.enter_context(tc.tile_pool(name="accp", bufs=4))
        # lgx/lgy: 2 banks each, single-buffered -> X/Y ping-pong
        plg = ctx.enter_context(tc.tile_pool(name="plg", bufs=1, space="PSUM"))
        # ctx accumulators + wo/proj scratch, 1 bank each, double-buffered
        pacc = ctx.enter_context(tc.tile_pool(name="pacc", bufs=2, space="PSUM"))

        wq_sb = const.tile([128, 2, NDC, 128], BF16, name="wq_sb")
        wk_sb = const.tile([128, 2, NDC, 128], BF16, name="wk_sb")
        wv_sb = const.tile([128, NDC, HD], BF16, name="wv_sb")
        onesP = const.tile([128, HPG], BF16, name="onesP")
        wo_sb = const.tile([128, 2, D], BF16, name="wo_sb")
        bq_sb = const.tile([128, 2], F32, name="bq_sb")
        bk_sb = const.tile([128, 2], F32, name="bk_sb")
        xT_sb = [const.tile([128, NDC, SP], BF16, name=f"xT{q}") for q in range(NQS)]
        KT_sb = const.tile([128, 2, S], BF16, name="KT_sb")
        V_sb = const.tile([128, NSC, HD], BF16, name="V_sb")

        # input DMAs: host pre-tiles everything to the exact SBUF layout,
        # so each tensor is one flat contiguous transfer. All inputs ride
        # one dynamic queue in emission order, so the order below is the
        # arrival order the prologue compute is scheduled against. The
        # m=0 halves of W_k/W_q come first: job 0 only needs those.
        nc.sync.dma_start(bk_sb[:], bk_e.ap())
        nc.sync.dma_start(bq_sb[:], bq_e.ap())
        nc.sync.dma_start(onesP[:], ones_e.ap())
        nc.sync.dma_start(wk_sb[:, 0, 0:2, :], wk_e.ap()[:, 0, 0:2, :])
        nc.sync.dma_start(xT_sb[0][:, 0:2, :], xT_e.ap()[0, :, 0:2, :])
        nc.sync.dma_start(wk_sb[:, 0, 2:, :], wk_e.ap()[:, 0, 2:, :])
        nc.sync.dma_start(xT_sb[0][:, 2:, :], xT_e.ap()[0, :, 2:, :])
        nc.sync.dma_start(wq_sb[:, 0], wq_e.ap()[:, 0])
        nc.sync.dma_start(wv_sb[:], wv_e.ap())
        nc.sync.dma_start(wq_sb[:, 1], wq_e.ap()[:, 1])
        nc.sync.dma_start(wk_sb[:, 1], wk_e.ap()[:, 1])
        for q in range(1, NQS):
            nc.sync.dma_start(xT_sb[q][:], xT_e.ap()[q])
        nc.sync.dma_start(wo_sb[:], wo_e.ap())

        # K^T projection group: KT[128 (2 heads), m, s]
        def emit_kproj_group(m, q):
            ps = pacc.tile([128, SP], F32, tag="wo", name="kps")
            for c in range(NDC):
                nc.tensor.matmul(
                    ps[:], wk_sb[:, m, c, :],
                    xT_sb[q][:, c, :],
                    start=(c == 0), stop=(c == NDC - 1))
            nc.vector.tensor_scalar(
                KT_sb[:, m, q * SP:(q + 1) * SP], ps[:],
                bk_sb[:, m:m + 1], None, ADD)

        # V projection, natural layout [s-chunk, 4*64]
        def emit_vproj_group(sc):
            q, si = divmod(sc, SI)
            ps = pacc.tile([128, HD], F32, tag="wo", name="vps")
            for c in range(NDC):
                nc.tensor.matmul(
                    ps[:], xT_sb[q][:, c, si * 128:(si + 1) * 128],
                    wv_sb[:, c, :],
                    start=(c == 0), stop=(c == NDC - 1))
            nc.vector.tensor_copy(V_sb[:, sc, :], ps[:])

        # last-span W_o is two-pass: m0 partials land in these persistent
        # SBUF tiles mid-span, only m1 + add + DMA remain for the epilogue
        woa = [const.tile([128, SP], F32, name=f"woa{i}")
               for i in range(SI * (D // SP))]

        def make_qproj_parts(QTn, qsrc, m, nparts=2):
            cell = []
            step = NDC // nparts
            def part(p):
                def run():
                    if p == 0:
                        cell.append(pacc.tile([128, SP], F32, tag="wo",
                                              name="qps"))
                    ps = cell[0]
                    for c in range(p * step, (p + 1) * step):
                        nc.tensor.matmul(
                            ps[:], wq_sb[:, m, c, :],
                            xT_sb[qsrc][:, c, :],
                            start=(c == 0), stop=(c == NDC - 1))
                    if p == nparts - 1:
                        nc.vector.tensor_scalar(
                            QTn[:, m, :], ps[:], bq_sb[:, m:m + 1], None, ADD)
                return run
            return [part(p) for p in range(nparts)]

        def emit_qproj_group(QTn, qsrc, m):
            for run in make_qproj_parts(QTn, qsrc, m, nparts=1):
                run()

        # Logits for one (pair, chunk): the two heads' K=64 matmuls go to
        # PE row groups 0/64 back-to-back (concurrent in HW), then one
        # [128, 1024] exp covers both heads.
        def emit_lg_pair(QT, p, scp, j, tag):
            lg = plg.tile([128, 2 * SP], F32, tag=tag, name="lg")
            sc = 2 * scp + j
            for hh in range(2):
                r = hh * 64
                nc.tensor.matmul(
                    lg[:, hh * SP:(hh + 1) * SP],
                    KT_sb[r:r + 64, p, sc * 128:(sc + 1) * 128],
                    QT[r:r + 64, p, :],
                    start=True, stop=True)
            pt = ptp.tile([128, 2 * SP], BF16, name="pt")
            nc.scalar.activation(pt[:], lg[:], EXP, scale=float(SCALE))
            return pt

        # ctx for one pair job: the two heads are PE col-tiles writing
        # partitions 0-63 / 64-127 of one shared psum bank (concurrent in
        # HW), and the softmax denominators accumulate on the DVE in bf16.
        def emit_ctx_pair(CT, cps_by_p, acc_by_p, p, scp, ptX, ptY):
            if scp == 0:
                cps_by_p[p] = pacc.tile([128, SP], F32, tag="ctx",
                                        name="cps")
                acc_by_p[p] = (accp.tile([128, SP], BF16, name="accA"),
                               accp.tile([128, SP], BF16, name="accB"))
            cps = cps_by_p[p]
            accA, accB = acc_by_p[p]
            for j, pt in ((0, ptX), (1, ptY)):
                sc = 2 * scp + j
                for hh in range(2):
                    h = 2 * p + hh
                    nc.tensor.matmul(
                        cps[hh * 64:(hh + 1) * 64, :],
                        V_sb[:, sc, h * DK:(h + 1) * DK],
                        pt[:, hh * SP:(hh + 1) * SP],
                        start=(sc == 0), stop=(sc == NSC - 1))
            for hh, acc in ((0, accA), (1, accB)):
                sl = slice(hh * SP, (hh + 1) * SP)
                if scp == 0:
                    nc.vector.tensor_tensor(
                        acc[:], ptX[:, sl], ptY[:, sl], ADD)
                else:
                    nc.vector.tensor_tensor(acc[:], acc[:], ptX[:, sl], ADD)
                    nc.vector.tensor_tensor(acc[:], acc[:], ptY[:, sl], ADD)
            if scp == NSC // 2 - 1:
                # both reduce+recip+broadcast parts run before either
                # multiply; all at +1 so every CT write is emitted before
                # any consumer (wo from jidx>=2, passa from jidx>=10)
                na = emit_norm(CT, 2 * p, cps, accA)
                nb = emit_norm(CT, 2 * p + 1, cps, accB)
                return [(1, na[0]), (1, nb[0]), (1, na[1]), (1, nb[1])]
            return None

        def emit_norm(CT, h, cps, acc):
            # denominators: PE partition-reduce of the bf16 accumulator
            # (ones lhsT, M=4 to stay off degenerate-shape paths), then
            # DVE reciprocal + gpsimd partition-broadcast a job ahead of
            # the multiply, keeping ~1us latency off the W_o path.
            cell = []
            def run_recip():
                nt = pacc.tile([68, SP], F32, tag="wo", name="nt")
                nc.tensor.matmul(nt[64:68, :], onesP[:, 0:4], acc[:],
                                 start=True, stop=True)
                sm = smp.tile([1, SP], F32, name="sm")
                nc.vector.tensor_copy(sm[:], nt[64:65, :])
                rc = smp.tile([1, SP], F32, name="rc")
                nc.vector.reciprocal_approx_fast(rc[:], sm[:])
                bc = smp.tile([64, SP], F32, name="bc")
                nc.gpsimd.partition_broadcast(bc[:], rc[:])
                cell.append(bc)
            def run_rest():
                m, hh = divmod(h, 2)
                r = hh * 64
                nc.vector.tensor_tensor(
                    CT[r:r + 64, m, :], cps[r:r + 64, :], cell[0][:], MULT)
            return run_recip, run_rest

        def make_wo_ops(q, CT):
            # each (si, dh) group split into two single-matmul halves so the
            # interleave never adds more than one extra matmul per period
            ops = []
            for si in range(SI):
                sc = SI * q + si
                for dh in range(D // SP):
                    cell = []
                    def half_a(si=si, dh=dh, CT=CT, cell=cell):
                        cell.append(pacc.tile([128, SP], F32, tag="wo",
                                              name="po"))
                        nc.tensor.matmul(
                            cell[0][:], CT[:, 0, si * 128:(si + 1) * 128],
                            wo_sb[:, 0, dh * SP:(dh + 1) * SP],
                            start=True, stop=False)
                    def half_b(si=si, sc=sc, dh=dh, CT=CT, cell=cell):
                        po = cell[0]
                        nc.tensor.matmul(
                            po[:], CT[:, 1, si * 128:(si + 1) * 128],
                            wo_sb[:, 1, dh * SP:(dh + 1) * SP],
                            start=False, stop=True)
                        ob = obp.tile([128, SP], F32, name="ob")
                        nc.vector.tensor_copy(ob[:], po[:])
                        nc.sync.dma_start(
                            out_e.ap()[sc * 128:(sc + 1) * 128,
                                       dh * SP:(dh + 1) * SP], ob[:])
                    ops.append(half_a)
                    ops.append(half_b)
            return ops

        # prologue: only what span-0 job 0 needs -- K^T (m=0, span 0) and
        # Q^T m=0. Everything else (remaining K^T, all of V, Q^T m=1)
        # interleaves into span 0's jobs just-in-time against DMA arrival.
        emit_kproj_group(0, 0)

        JPS = 2 * (NSC // 2)            # jobs per span: (pair, scp)
        QT_t = {0: qpool.tile([128, 2, SP], BF16, name="QT")}
        emit_qproj_group(QT_t[0], 0, 0)
        CT_t = {}
        cps_t = {}
        acc_t = {}
        LAG = 2
        pend_q = []                     # [(q, p, scp, ptX, ptY)]
        deferred = []                   # (due_gidx, closure)
        wo_queue = []
        # span-0 fill work, per-job lists: each producer is emitted at
        # latest one job before (or, for KT, in the same job but ahead
        # of) its first consumer, roughly in DMA-arrival order.
        vp = lambda sc: (lambda: emit_vproj_group(sc))
        kp = lambda q: (lambda: emit_kproj_group(1, q))
        k0 = lambda q: (lambda: emit_kproj_group(0, q))
        fill_sched = [
            [vp(0), vp(1), lambda: emit_qproj_group(QT_t[0], 0, 1)],
            [k0(1), vp(2)],
            [vp(3), vp(4)],
            [k0(2), vp(5)],
            [vp(6), vp(7)],
            [k0(3), vp(8), vp(9)],
            [vp(10), vp(11)],
            [kp(0), vp(12), vp(13)],
            [vp(14), vp(15)],
            [kp(1)],
            [kp(2)],
            [kp(3)],
        ]

        for q in range(NQS):
            CT_t[q] = cpool.tile([128, 2, SP], BF16, name="CT")
            cps_t[q] = {}
            acc_t[q] = {}
            qpart_queue = []
            if q + 1 < NQS:
                QT_t[q + 1] = qpool.tile([128, 2, SP], BF16, name="QT")
                qpart_queue = (make_qproj_parts(QT_t[q + 1], q + 1, 0,
                                                nparts=NDC)
                               + make_qproj_parts(QT_t[q + 1], q + 1, 1,
                                                  nparts=NDC))

            # in the last span, pair 1 runs first so its heads' norms
            # (CT[:, 1, :]) complete mid-span -- pass-a covers the m=1
            # W_o rows then, leaving only m=0 + add + DMA for the tail
            p_of = (lambda jj: 1 - jj // (NSC // 2)) if q == NQS - 1 \
                else (lambda jj: jj // (NSC // 2))
            pa_m = 1
            passa = []
            if q == NQS - 1:
                def mk_passa(i, si, dh, CT=CT_t[q]):
                    def run():
                        po = pacc.tile([128, SP], F32, tag="wo", name="po")
                        nc.tensor.matmul(
                            po[:], CT[:, pa_m, si * 128:(si + 1) * 128],
                            wo_sb[:, pa_m, dh * SP:(dh + 1) * SP],
                            start=True, stop=True)
                        nc.vector.tensor_copy(woa[i][:], po[:])
                    return run
                passa = [mk_passa(si * (D // SP) + dh, si, dh)
                         for si in range(SI) for dh in range(D // SP)]

            for jidx in range(JPS):
                gidx = q * JPS + jidx
                p, scp = p_of(jidx), jidx % (NSC // 2)
                # norms first: they free the ctx psum slots this job's
                # ctx matmuls may be waiting to reuse
                for ent in list(deferred):
                    if ent[0] <= gidx:
                        ent[1]()
                        deferred.remove(ent)
                ptX = emit_lg_pair(QT_t[q], p, scp, 0, "lgx")
                ptY = emit_lg_pair(QT_t[q], p, scp, 1, "lgy")
                pend_q.append((q, p, scp, ptX, ptY))
                # span-0 producers: after this job's logits (which only
                # read earlier products) but before the lagged ctx
                if q == 0 and jidx < len(fill_sched):
                    for op in fill_sched[jidx]:
                        op()
                if len(pend_q) > LAG:
                    ent0 = pend_q.pop(0)
                    nrm = emit_ctx_pair(CT_t[ent0[0]], cps_t[ent0[0]],
                                        acc_t[ent0[0]], *ent0[1:])
                    if nrm is not None:
                        for delta, closure in nrm:
                            deferred.append((gidx + delta, closure))
                if wo_queue and jidx >= 2:
                    wo_queue.pop(0)()
                    if wo_queue and jidx % 2 == 0:
                        wo_queue.pop(0)()
                if qpart_queue and jidx >= 4:
                    qpart_queue.pop(0)()
                    if qpart_queue:
                        qpart_queue.pop(0)()
                if passa and jidx >= 10:
                    passa.pop(0)()
                    if passa:
                        passa.pop(0)()
            if q < NQS - 1:
                wo_queue.extend(make_wo_ops(q, CT_t[q]))

        # epilogue: drain the pipeline
        norms = []
        for ent0 in pend_q:
            nrm = emit_ctx_pair(CT_t[ent0[0]], cps_t[ent0[0]],
                                acc_t[ent0[0]], *ent0[1:])
            if nrm is not None:
                norms.extend(nrm)
        for ent in sorted(deferred, key=lambda e: e[0]):
            ent[1]()
        for _, closure in sorted(norms, key=lambda e: e[0]):
            closure()
        for op in wo_queue:
            op()
        for op in passa:
            op()
        CTl = CT_t[NQS - 1]
        ep_m = 1 - pa_m
        for si in range(SI):
            sc = SI * (NQS - 1) + si
            for dh in range(D // SP):
                i = si * (D // SP) + dh
                po = pacc.tile([128, SP], F32, tag="wo", name="po")
                nc.tensor.matmul(
                    po[:], CTl[:, ep_m, si * 128:(si + 1) * 128],
                    wo_sb[:, ep_m, dh * SP:(dh + 1) * SP],
                    start=True, stop=True)
                ob = obp.tile([128, SP], F32, name="ob")
                nc.vector.tensor_tensor(ob[:], woa[i][:], po[:], ADD)
                nc.sync.dma_start(
                    out_e.ap()[sc * 128:(sc + 1) * 128,
                               dh * SP:(dh + 1) * SP], ob[:])

    nc.compile()
    return nc


_NC_CACHE = {}


def get_nc(S):
    if S not in _NC_CACHE:
        _NC_CACHE[S] = build(S)
    return _NC_CACHE[S]


def make_in_maps(x, W_q, b_q, W_k, b_k, W_v, b_v, W_o, b_o):
    B, S, _ = x.shape
    bf = ml_dtypes.bfloat16
    in_maps = []
    for core in range(NCORES):
        b, g = divmod(core, GPB)
        sl = slice(HD * g, HD * (g + 1))
        def wtile(w):
            # [D, N] -> [128, D//128, N] partition-major chunk layout
            return np.ascontiguousarray(
                np.asarray(w).reshape(D // 128, 128, -1).transpose(1, 0, 2))
        def wtile_m(w):
            # [D, 256] -> [128, 2(m), D//128, 128] m-major chunk layout
            return np.ascontiguousarray(
                np.asarray(w).reshape(D // 128, 128, 2, 128)
                .transpose(1, 2, 0, 3))
        in_maps.append({
            "xT": np.ascontiguousarray(
                np.asarray(x[b]).T.reshape(D // 128, 128, S // SP, SP)
                .transpose(2, 1, 0, 3)).astype(bf),
            "wq": wtile_m(W_q[:, sl]).astype(bf),
            "wk": wtile_m(W_k[:, sl]).astype(bf),
            "wv": wtile(W_v[:, sl]).astype(bf),
            "wo": np.ascontiguousarray(
                np.asarray(W_o[sl, :]).reshape(2, 128, D)
                .transpose(1, 0, 2)).astype(bf),
            "bq": np.ascontiguousarray(
                np.asarray(b_q[sl]).reshape(2, 128).T).astype(np.float32),
            "bk": np.ascontiguousarray(
                np.asarray(b_k[sl]).reshape(2, 128).T).astype(np.float32),
            "ones": np.ones((128, HPG), dtype=bf),
        })
    return in_maps


def unshard(results, x, W_o, b_v, b_o):
    B, S, _ = x.shape
    out = np.zeros((B, S, D), np.float32)
    for core in range(NCORES):
        b = core // GPB
        out[b] += results[core]["out"]
    const = np.asarray(b_v).astype(np.float64) @ np.asarray(W_o).astype(np.float64)
    const += np.asarray(b_o).astype(np.float64)
    out += const.astype(np.float32)[None, None, :]
    return out


def run(inputs, trace=False):
    x = np.asarray(inputs["x"])
    nc = get_nc(x.shape[1])
    in_maps = make_in_maps(
        x, inputs["W_q"], inputs["b_q"], inputs["W_k"], inputs["b_k"],
        inputs["W_v"], inputs["b_v"], inputs["W_o"], inputs["b_o"])
    def attempt():
        res = run_bass_kernel_spmd(
            nc, in_maps, core_ids=list(range(NCORES)), trace=trace)
        # force materialization here: PJRT surfaces device errors lazily
        res.results = [{k: np.asarray(v) for k, v in r.items()}
                       for r in res.results]
        return res
    try:
        res = attempt()
    except Exception:
        # transient device errors (e.g. NRT_EXEC_UNIT_UNRECOVERABLE) clear
        # on re-execution of the same NEFF
        res = attempt()
    out = unshard(res.results, x, inputs["W_o"], inputs["b_v"], inputs["b_o"])
    return out, res


def kernel(**inputs):
    out, _ = run(inputs, trace=False)
    return out


# revision 39
# speedup vs baseline: 1.0425x; 1.0164x over previous
"""Multi-head attention (B=2, S=2048, D=1024, H=16) on 8 TRN2 NeuronCores.

Sharding: tensor-parallel over heads x data-parallel over batch.
Core c handles batch b = c//4, head group g = c%4 (4 heads, 256 cols).
W_q/W_k/W_v are split column-wise per group, W_o row-wise; each core
produces a partial [S, D] output, reduced on the host (the W_o
contraction is a pure sum over head groups; b_v/b_o folded in on host).

Device kernel (per core), all matmuls bf16 with fp32 PSUM accumulation:
  - K^T, Q^T projections in transposed layout [dk*2, S] (lhsT = W cols,
    rhs = x^T), V in natural layout [S, dk*4+ones] (lhsT = x^T chunks).
  - scores computed transposed: ST[k,q] = (K^T chunk)^T-matmul vs Q^T.
    The two heads of a pair (rows 0-63 / 64-127 of KT/QT) are issued
    back-to-back as PE row-tiles (tile_position (0,0) / (64,0)) so the
    hardware streams them concurrently -- logits cost ~1 matmul slot
    per head pair instead of 2.
  - softmax without max-subtraction (logits are O(5), exp is safe):
    exp on ACT straight out of PSUM with scale=1/sqrt(dk), one
    [128, 1024] activation per (pair, chunk) covering both heads.
    lgx/lgy psum tiles ping-pong so ACT back-pressure never blocks
    the PE fill of the next chunk.
  - ctx^T[dk+1, q] accumulated over k-chunks with an ones-augmented V
    (row dk = softmax denominators), normalized via DVE with a gpsimd
    partition-broadcast of the reciprocals.
  - out partial = ctx^T-chunks @ W_o rows, accumulated over the 2
    128-row chunks of the group's 256 W_o rows.
"""

import numpy as np
import ml_dtypes
from contextlib import ExitStack

import concourse.bass as bass
import concourse.tile as tile
from concourse import bacc, mybir
from concourse.bass_utils import run_bass_kernel_spmd

BF16 = mybir.dt.bfloat16
F32 = mybir.dt.float32

D = 1024            # model dim
H = 16              # heads
DK = 64             # head dim
NCORES = 8
GPB = 4             # head groups per batch (= cores per batch)
HPG = H // GPB      # 4 heads per core
HD = HPG * DK       # 256 cols per group
HAUG = DK + 1       # 65: head block width in augmented-V layout
SP = 512            # q-span / free-dim tile
SCALE = 1.0 / np.sqrt(DK)


def build(S, debug_dump=False):
    NQS = S // SP       # q spans
    NSC = S // 128      # sequence chunks (k side)
    NDC = D // 128      # model-dim chunks
    SI = SP // 128      # s-chunks per q-span

    nc = bacc.Bacc("TRN2", target_bir_lowering=False, debug=False)
    HA = HPG * HAUG     # 260: augmented V width
    xT_e = nc.dram_tensor("xT", [S // SP, 128, D // 128, SP], BF16, kind="ExternalInput")
    wq_e = nc.dram_tensor("wq", [128, 2, D // 128, 128], BF16, kind="ExternalInput")
    wk_e = nc.dram_tensor("wk", [128, 2, D // 128, 128], BF16, kind="ExternalInput")
    wv_e = nc.dram_tensor("wv", [128, D // 128, HD], BF16, kind="ExternalInput")
    wo_e = nc.dram_tensor("wo", [128, 2, D], BF16, kind="ExternalInput")
    bq_e = nc.dram_tensor("bq", [128, 2], F32, kind="ExternalInput")
    bk_e = nc.dram_tensor("bk", [128, 2], F32, kind="ExternalInput")
    ones_e = nc.dram_tensor("ones", [128, HPG], BF16, kind="ExternalInput")
    out_e = nc.dram_tensor("out", [S, D], F32, kind="ExternalOutput")

    ADD = mybir.AluOpType.add
    MULT = mybir.AluOpType.mult
    EXP = mybir.ActivationFunctionType.Exp

    with tile.TileContext(nc) as tc, ExitStack() as ctx:
        const = ctx.enter_context(tc.tile_pool(name="const", bufs=1))
        qpool = ctx.enter_context(tc.tile_pool(name="qpool", bufs=2))
        cpool = ctx.enter_context(tc.tile_pool(name="cpool", bufs=2))
        ptp = ctx.enter_context(tc.tile_pool(name="ptp", bufs=6))
        obp = ctx.enter_context(tc.tile_pool(name="obp", bufs=4))
        smp = ctx.enter_context(tc.tile_pool(name="smp", bufs=3))
        accp = ctx.enter_context(tc.tile_pool(name="accp", bufs=4))
        # lgx/lgy: 2 banks each, single-buffered -> X/Y ping-pong
        plg = ctx.enter_context(tc.tile_pool(name="plg", bufs=1, space="PSUM"))
        # ctx accumulators + wo/proj scratch, 1 bank each, double-buffered
        pacc = ctx.enter_context(tc.tile_pool(name="pacc", bufs=2, space="PSUM"))

        wq_sb = const.tile([128, 2, NDC, 128], BF16, name="wq_sb")
        wk_sb = const.tile([128, 2, NDC, 128], BF16, name="wk_sb")
        wv_sb = const.tile([128, NDC, HD], BF16, name="wv_sb")
        onesP = const.tile([128, HPG], BF16, name="onesP")
        wo_sb = const.tile([128, 2, D], BF16, name="wo_sb")
        bq_sb = const.tile([128, 2], F32, name="bq_sb")
        bk_sb = const.tile([128, 2], F32, name="bk_sb")
        xT_sb = [const.tile([128, NDC, SP], BF16, name=f"xT{q}") for q in range(NQS)]
        KT_sb = const.tile([128, 2, S], BF16, name="KT_sb")
        V_sb = const.tile([128, NSC, HD], BF16, name="V_sb")

        # input DMAs: host pre-tiles everything to the exact SBUF layout,
        # so each tensor is one flat contiguous transfer. All inputs ride
        # one dynamic queue in emission order, so the order below is the
        # arrival order the prologue compute is scheduled against. The
        # m=0 halves of W_k/W_q come first: job 0 only needs those.
        nc.sync.dma_start(bk_sb[:], bk_e.ap())
        nc.sync.dma_start(bq_sb[:], bq_e.ap())
        nc.sync.dma_start(onesP[:], ones_e.ap())
        nc.sync.dma_start(wk_sb[:, 0, 0:2, :], wk_e.ap()[:, 0, 0:2, :])
        nc.sync.dma_start(xT_sb[0][:, 0:2, :], xT_e.ap()[0, :, 0:2, :])
        nc.sync.dma_start(wk_sb[:, 0, 2:, :], wk_e.ap()[:, 0, 2:, :])
        nc.sync.dma_start(xT_sb[0][:, 2:, :], xT_e.ap()[0, :, 2:, :])
        nc.sync.dma_start(wq_sb[:, 0], wq_e.ap()[:, 0])
        nc.sync.dma_start(wv_sb[:], wv_e.ap())
        nc.sync.dma_start(wq_sb[:, 1], wq_e.ap()[:, 1])
        nc.sync.dma_start(wk_sb[:, 1], wk_e.ap()[:, 1])
        for q in range(1, NQS):
            nc.sync.dma_start(xT_sb[q][:], xT_e.ap()[q])
        nc.sync.dma_start(wo_sb[:], wo_e.ap())

        # K^T projection group: KT[128 (2 heads), m, s]
        def emit_kproj_group(m, q):
            ps = pacc.tile([128, SP], F32, tag="wo", name="kps")
            for c in range(NDC):
                nc.tensor.matmul(
                    ps[:], wk_sb[:, m, c, :],
                    xT_sb[q][:, c, :],
                    start=(c == 0), stop=(c == NDC - 1))
            nc.vector.tensor_scalar(
                KT_sb[:, m, q * SP:(q + 1) * SP], ps[:],
                bk_sb[:, m:m + 1], None, ADD)

        # V projection, natural layout [s-chunk, 4*64]
        def emit_vproj_group(sc):
            q, si = divmod(sc, SI)
            ps = pacc.tile([128, HD], F32, tag="wo", name="vps")
            for c in range(NDC):
                nc.tensor.matmul(
                    ps[:], xT_sb[q][:, c, si * 128:(si + 1) * 128],
                    wv_sb[:, c, :],
                    start=(c == 0), stop=(c == NDC - 1))
            nc.vector.tensor_copy(V_sb[:, sc, :], ps[:])

        # last-span W_o is two-pass: m0 partials land in these persistent
        # SBUF tiles mid-span, only m1 + add + DMA remain for the epilogue
        woa = [const.tile([128, SP], F32, name=f"woa{i}")
               for i in range(SI * (D // SP))]

        def make_qproj_parts(QTn, qsrc, m, nparts=2):
            cell = []
            step = NDC // nparts
            def part(p):
                def run():
                    if p == 0:
                        cell.append(pacc.tile([128, SP], F32, tag="wo",
                                              name="qps"))
                    ps = cell[0]
                    for c in range(p * step, (p + 1) * step):
                        nc.tensor.matmul(
                            ps[:], wq_sb[:, m, c, :],
                            xT_sb[qsrc][:, c, :],
                            start=(c == 0), stop=(c == NDC - 1))
                    if p == nparts - 1:
                        nc.vector.tensor_scalar(
                            QTn[:, m, :], ps[:], bq_sb[:, m:m + 1], None, ADD)
                return run
            return [part(p) for p in range(nparts)]

        def emit_qproj_group(QTn, qsrc, m):
            for run in make_qproj_parts(QTn, qsrc, m, nparts=1):
                run()

        # Logits for one (pair, chunk): the two heads' K=64 matmuls go to
        # PE row groups 0/64 back-to-back (concurrent in HW), then one
        # [128, 1024] exp covers both heads.
        def emit_lg_pair(QT, p, scp, j, tag):
            lg = plg.tile([128, 2 * SP], F32, tag=tag, name="lg")
            sc = 2 * scp + j
            for hh in range(2):
                r = hh * 64
                nc.tensor.matmul(
                    lg[:, hh * SP:(hh + 1) * SP],
                    KT_sb[r:r + 64, p, sc * 128:(sc + 1) * 128],
                    QT[r:r + 64, p, :],
                    start=True, stop=True)
            pt = ptp.tile([128, 2 * SP], BF16, name="pt")
            nc.scalar.activation(pt[:], lg[:], EXP, scale=float(SCALE))
            return pt

        # ctx for one pair job: the two heads are PE col-tiles writing
        # partitions 0-63 / 64-127 of one shared psum bank (concurrent in
        # HW), and the softmax denominators accumulate on the DVE in bf16.
        def emit_ctx_pair(CT, cps_by_p, acc_by_p, p, scp, ptX, ptY):
            if scp == 0:
                cps_by_p[p] = pacc.tile([128, SP], F32, tag="ctx",
                                        name="cps")
                acc_by_p[p] = (accp.tile([128, SP], BF16, name="accA"),
                               accp.tile([128, SP], BF16, name="accB"))
            cps = cps_by_p[p]
            accA, accB = acc_by_p[p]
            for j, pt in ((0, ptX), (1, ptY)):
                sc = 2 * scp + j
                for hh in range(2):
                    h = 2 * p + hh
                    nc.tensor.matmul(
                        cps[hh * 64:(hh + 1) * 64, :],
                        V_sb[:, sc, h * DK:(h + 1) * DK],
                        pt[:, hh * SP:(hh + 1) * SP],
                        start=(sc == 0), stop=(sc == NSC - 1))
            def run_adds():
                # denominator partials -- emitted at the end of the job
                # body so these bulk DVE ops never sit ahead of critical
                # DVE work (CT mults, V/ob copies) in the in-order queue
                for hh, acc in ((0, accA), (1, accB)):
                    sl = slice(hh * SP, (hh + 1) * SP)
                    if scp == 0:
                        nc.vector.tensor_tensor(
                            acc[:], ptX[:, sl], ptY[:, sl], ADD)
                    else:
                        nc.vector.tensor_tensor(
                            acc[:], acc[:], ptX[:, sl], ADD)
                        nc.vector.tensor_tensor(
                            acc[:], acc[:], ptY[:, sl], ADD)
            if scp == NSC // 2 - 1:
                # both reduce+recip+broadcast parts run before either
                # multiply; all at +1 so every CT write is emitted before
                # any consumer (wo from jidx>=2, passa from jidx>=10)
                na = emit_norm(CT, 2 * p, cps, accA)
                nb = emit_norm(CT, 2 * p + 1, cps, accB)
                return ([(1, na[0]), (1, nb[0]), (1, na[1]), (1, nb[1])],
                        run_adds)
            return (None, run_adds)

        def emit_norm(CT, h, cps, acc):
            # denominators: PE partition-reduce of the bf16 accumulator
            # (ones lhsT, M=4 to stay off degenerate-shape paths), then
            # DVE reciprocal + gpsimd partition-broadcast a job ahead of
            # the multiply, keeping ~1us latency off the W_o path.
            cell = []
            def run_recip():
                nt = pacc.tile([68, SP], F32, tag="wo", name="nt")
                nc.tensor.matmul(nt[64:68, :], onesP[:, 0:4], acc[:],
                                 start=True, stop=True)
                sm = smp.tile([1, SP], F32, name="sm")
                nc.vector.tensor_copy(sm[:], nt[64:65, :])
                rc = smp.tile([1, SP], F32, name="rc")
                nc.vector.reciprocal_approx_fast(rc[:], sm[:])
                bc = smp.tile([64, SP], F32, name="bc")
                nc.gpsimd.partition_broadcast(bc[:], rc[:])
                cell.append(bc)
            def run_rest():
                m, hh = divmod(h, 2)
                r = hh * 64
                nc.vector.tensor_tensor(
                    CT[r:r + 64, m, :], cps[r:r + 64, :], cell[0][:], MULT)
            return run_recip, run_rest

        def make_wo_ops(q, CT):
            # each (si, dh) group split into two single-matmul halves so the
            # interleave never adds more than one extra matmul per period
            ops = []
            for si in range(SI):
                sc = SI * q + si
                for dh in range(D // SP):
                    cell = []
                    def half_a(si=si, dh=dh, CT=CT, cell=cell):
                        cell.append(pacc.tile([128, SP], F32, tag="wo",
                                              name="po"))
                        nc.tensor.matmul(
                            cell[0][:], CT[:, 0, si * 128:(si + 1) * 128],
                            wo_sb[:, 0, dh * SP:(dh + 1) * SP],
                            start=True, stop=False)
                    def half_b(si=si, sc=sc, dh=dh, CT=CT, cell=cell):
                        po = cell[0]
                        nc.tensor.matmul(
                            po[:], CT[:, 1, si * 128:(si + 1) * 128],
                            wo_sb[:, 1, dh * SP:(dh + 1) * SP],
                            start=False, stop=True)
                        ob = obp.tile([128, SP], F32, name="ob")
                        nc.vector.tensor_copy(ob[:], po[:])
                        nc.sync.dma_start(
                            out_e.ap()[sc * 128:(sc + 1) * 128,
                                       dh * SP:(dh + 1) * SP], ob[:])
                    ops.append(half_a)
                    ops.append(half_b)
            return ops

        # prologue: only what span-0 job 0 needs -- K^T (m=0, span 0) and
        # Q^T m=0. Everything else (remaining K^T, all of V, Q^T m=1)
        # interleaves into span 0's jobs just-in-time against DMA arrival.
        emit_kproj_group(0, 0)

        JPS = 2 * (NSC // 2)            # jobs per span: (pair, scp)
        QT_t = {0: qpool.tile([128, 2, SP], BF16, name="QT")}
        emit_qproj_group(QT_t[0], 0, 0)
        CT_t = {}
        cps_t = {}
        acc_t = {}
        LAG = 2
        pend_q = []                     # [(q, p, scp, ptX, ptY)]
        deferred = []                   # (due_gidx, closure)
        wo_queue = []
        # span-0 fill work, per-job lists: each producer is emitted at
        # latest one job before (or, for KT, in the same job but ahead
        # of) its first consumer, roughly in DMA-arrival order.
        vp = lambda sc: (lambda: emit_vproj_group(sc))
        kp = lambda q: (lambda: emit_kproj_group(1, q))
        k0 = lambda q: (lambda: emit_kproj_group(0, q))
        fill_sched = [
            [vp(0), vp(1), lambda: emit_qproj_group(QT_t[0], 0, 1)],
            [k0(1), vp(2)],
            [vp(3), vp(4)],
            [k0(2), vp(5)],
            [vp(6), vp(7)],
            [k0(3), vp(8), vp(9)],
            [vp(10), vp(11)],
            [kp(0), vp(12), vp(13)],
            [vp(14), vp(15)],
            [kp(1)],
            [kp(2)],
            [kp(3)],
        ]

        for q in range(NQS):
            CT_t[q] = cpool.tile([128, 2, SP], BF16, name="CT")
            cps_t[q] = {}
            acc_t[q] = {}
            qpart_queue = []
            if q + 1 < NQS:
                QT_t[q + 1] = qpool.tile([128, 2, SP], BF16, name="QT")
                qpart_queue = (make_qproj_parts(QT_t[q + 1], q + 1, 0,
                                                nparts=NDC)
                               + make_qproj_parts(QT_t[q + 1], q + 1, 1,
                                                  nparts=NDC))

            # in the last span, pair 1 runs first so its heads' norms
            # (CT[:, 1, :]) complete mid-span -- pass-a covers the m=1
            # W_o rows then, leaving only m=0 + add + DMA for the tail
            p_of = (lambda jj: 1 - jj // (NSC // 2)) if q == NQS - 1 \
                else (lambda jj: jj // (NSC // 2))
            pa_m = 1
            passa = []
            if q == NQS - 1:
                def mk_passa(i, si, dh, CT=CT_t[q]):
                    def run():
                        po = pacc.tile([128, SP], F32, tag="wo", name="po")
                        nc.tensor.matmul(
                            po[:], CT[:, pa_m, si * 128:(si + 1) * 128],
                            wo_sb[:, pa_m, dh * SP:(dh + 1) * SP],
                            start=True, stop=True)
                        nc.vector.tensor_copy(woa[i][:], po[:])
                    return run
                passa = [mk_passa(si * (D // SP) + dh, si, dh)
                         for si in range(SI) for dh in range(D // SP)]

            for jidx in range(JPS):
                gidx = q * JPS + jidx
                p, scp = p_of(jidx), jidx % (NSC // 2)
                # norms first: they free the ctx psum slots this job's
                # ctx matmuls may be waiting to reuse
                for ent in list(deferred):
                    if ent[0] <= gidx:
                        ent[1]()
                        deferred.remove(ent)
                ptX = emit_lg_pair(QT_t[q], p, scp, 0, "lgx")
                ptY = emit_lg_pair(QT_t[q], p, scp, 1, "lgy")
                pend_q.append((q, p, scp, ptX, ptY))
                # span-0 producers: after this job's logits (which only
                # read earlier products) but before the lagged ctx
                if q == 0 and jidx < len(fill_sched):
                    for op in fill_sched[jidx]:
                        op()
                # last span drains the ctx lag early so the epilogue only
                # has one pending job left
                eff_lag = 1 if (q == NQS - 1 and jidx == JPS - 1) else LAG
                adds = []
                while len(pend_q) > eff_lag:
                    ent0 = pend_q.pop(0)
                    nrm, run_adds = emit_ctx_pair(
                        CT_t[ent0[0]], cps_t[ent0[0]],
                        acc_t[ent0[0]], *ent0[1:])
                    adds.append(run_adds)
                    if nrm is not None:
                        for delta, closure in nrm:
                            deferred.append((gidx + delta, closure))
                if wo_queue and jidx >= 2:
                    wo_queue.pop(0)()
                    if wo_queue and jidx % 2 == 0:
                        wo_queue.pop(0)()
                if qpart_queue and jidx >= 4:
                    qpart_queue.pop(0)()
                    if qpart_queue:
                        qpart_queue.pop(0)()
                if passa and jidx >= 10:
                    passa.pop(0)()
                    if passa:
                        passa.pop(0)()
                for run_adds in adds:
                    run_adds()
            if q < NQS - 1:
                wo_queue.extend(make_wo_ops(q, CT_t[q]))

        # epilogue: drain the pipeline
        norms = []
        for ent0 in pend_q:
            nrm, run_adds = emit_ctx_pair(CT_t[ent0[0]], cps_t[ent0[0]],
                                          acc_t[ent0[0]], *ent0[1:])
            run_adds()
            if nrm is not None:
                norms.extend(nrm)
        for ent in sorted(deferred, key=lambda e: e[0]):
            ent[1]()
        for _, closure in sorted(norms, key=lambda e: e[0]):
            closure()
        for op in wo_queue:
            op()
        for op in passa:
            op()
        CTl = CT_t[NQS - 1]
        ep_m = 1 - pa_m
        for si in range(SI):
            sc = SI * (NQS - 1) + si
            for dh in range(D // SP):
                i = si * (D // SP) + dh
                po = pacc.tile([128, SP], F32, tag="wo", name="po")
                nc.tensor.matmul(
                    po[:], CTl[:, ep_m, si * 128:(si + 1) * 128],
                    wo_sb[:, ep_m, dh * SP:(dh + 1) * SP],
                    start=True, stop=True)
                ob = obp.tile([128, SP], F32, name="ob")
                nc.vector.tensor_tensor(ob[:], woa[i][:], po[:], ADD)
                nc.sync.dma_start(
                    out_e.ap()[sc * 128:(sc + 1) * 128,
                               dh * SP:(dh + 1) * SP], ob[:])

    nc.compile()
    return nc


_NC_CACHE = {}


def get_nc(S):
    if S not in _NC_CACHE:
        _NC_CACHE[S] = build(S)
    return _NC_CACHE[S]


def make_in_maps(x, W_q, b_q, W_k, b_k, W_v, b_v, W_o, b_o):
    B, S, _ = x.shape
    bf = ml_dtypes.bfloat16
    in_maps = []
    for core in range(NCORES):
        b, g = divmod(core, GPB)
        sl = slice(HD * g, HD * (g + 1))
        def wtile(w):
            # [D, N] -> [128, D//128, N] partition-major chunk layout
            return np.ascontiguousarray(
                np.asarray(w).reshape(D // 128, 128, -1).transpose(1, 0, 2))
        def wtile_m(w):
            # [D, 256] -> [128, 2(m), D//128, 128] m-major chunk layout
            return np.ascontiguousarray(
                np.asarray(w).reshape(D // 128, 128, 2, 128)
                .transpose(1, 2, 0, 3))
        in_maps.append({
            "xT": np.ascontiguousarray(
                np.asarray(x[b]).T.reshape(D // 128, 128, S // SP, SP)
                .transpose(2, 1, 0, 3)).astype(bf),
            "wq": wtile_m(W_q[:, sl]).astype(bf),
            "wk": wtile_m(W_k[:, sl]).astype(bf),
            "wv": wtile(W_v[:, sl]).astype(bf),
            "wo": np.ascontiguousarray(
                np.asarray(W_o[sl, :]).reshape(2, 128, D)
                .transpose(1, 0, 2)).astype(bf),
            "bq": np.ascontiguousarray(
                np.asarray(b_q[sl]).reshape(2, 128).T).astype(np.float32),
            "bk": np.ascontiguousarray(
                np.asarray(b_k[sl]).reshape(2, 128).T).astype(np.float32),
            "ones": np.ones((128, HPG), dtype=bf),
        })
    return in_maps


def unshard(results, x, W_o, b_v, b_o):
    B, S, _ = x.shape
    out = np.zeros((B, S, D), np.float32)
    for core in range(NCORES):
        b = core // GPB
        out[b] += results[core]["out"]
    const = np.asarray(b_v).astype(np.float64) @ np.asarray(W_o).astype(np.float64)
    const += np.asarray(b_o).astype(np.float64)
    out += const.astype(np.float32)[None, None, :]
    return out


def run(inputs, trace=False):
    x = np.asarray(inputs["x"])
    nc = get_nc(x.shape[1])
    in_maps = make_in_maps(
        x, inputs["W_q"], inputs["b_q"], inputs["W_k"], inputs["b_k"],
        inputs["W_v"], inputs["b_v"], inputs["W_o"], inputs["b_o"])
    def attempt():
        res = run_bass_kernel_spmd(
            nc, in_maps, core_ids=list(range(NCORES)), trace=trace)
        # force materialization here: PJRT surfaces device errors lazily
        res.results = [{k: np.asarray(v) for k, v in r.items()}
                       for r in res.results]
        return res
    try:
        res = attempt()
    except Exception:
        # transient device errors (e.g. NRT_EXEC_UNIT_UNRECOVERABLE) clear
        # on re-execution of the same NEFF
        res = attempt()
    out = unshard(res.results, x, inputs["W_o"], inputs["b_v"], inputs["b_o"])
    return out, res


def kernel(**inputs):
    out, _ = run(inputs, trace=False)
    return out
